# revision 13
# baseline (speedup 1.0000x reference)
"""Trainium2 Bass kernel for nn_CentroidLoss (BCE + sparse-centroid selem similarity).

Full inputs in, full (scalar) output out. Sharding: flattened voxel axis N
split contiguously across 8 cores (one D-slice each).

Math: loss = mean_c BCE(x_c, t_c) + 0.5*mean(sims[:3]) + 0.5*(1-sims[3]) where
sims_c = (1/n_cent) * sum_i cm_i * (sum_k w_k valid x_c[i+off_k]) / cnt_i.
The centroid mask cm is ~0.01% dense, so the double sum is re-associated as
dot(x_c, A) with A[j] = sum_{i,k: i+off_k=j} cm_i * w_k / cnt_i, a sparse
scatter computed on host from the mask (~82*243 scalar ops).

Device (per core): ONE packed DRAM tensor (128 partitions x 4802 f32 =
x[4ch f32] | A[f32] | bias{0,1} | t[4ch u8]) loaded via 4 partition-group
DMAs (DMA cost here is per-descriptor, so fewest+fattest rows win, split
across rings for parallelism). ScalarE computes ln(x), ln(1-x) (table
prewarmed); VectorE does 4 fused multiply+row-sum reductions (channels
combined — only channel-sums are needed); PE folds the (128,5) partials to
(5,1) with a ones-column matmul; host combines 8x5 partials into the loss.
CHAN_WEIGHTS are all 1.0, so only channel-summed BCE/dot terms are needed.
"""

import os
import numpy as np

import concourse.bass as bass
import concourse.mybir as mybir
from concourse.tile import TileContext
from concourse import bass_utils

# ---- hardcoded problem geometry ----
D, H, W3 = 8, 320, 320
N = D * H * W3                     # 819200
NCORES = 8
CHUNK = N // NCORES                # 102400
P = 128
F = CHUNK // P                     # 800
CH = 4
EPS = 1e-7
ETA = 0.5
PHI = 0.5

SELEM_SHAPE = (3, 9, 9)
CENTRE = (1, 4, 4)

# packed-row layout (f32 columns)
XW = CH * F                        # 3200: x, channel-major
AW = F                             # 800: A
BW = 2                             # bias 0.0, 1.0
TW = CH * F // 4                   # 800: t as u8 bytes in f32 words
WTOT = XW + AW + BW + TW           # 4802

_cache = {}


def _split_multi_waits(nc):
    """This walrus build rejects >1 sync-wait per instruction ("Too many sync
    wait commands"). Tile coalesces waits; redistribute extras onto NoOps
    inserted immediately before, on the same engine (engine blocks on each
    wait in turn — semantics preserved)."""
    n_split = 0
    for fn in nc.m.functions:
        for b in fn.blocks:
            insts = b.instructions
            i = 0
            while i < len(insts):
                inst = insts[i]
                si = getattr(inst, 'sync_info', None)
                if si is None or not si.on_wait or len(si.on_wait) <= 1:
                    i += 1
                    continue
                waits = list(si.on_wait)
                new_nops = [
                    mybir.InstNoOp(
                        name=f"{inst.name}-waitsplit-{k}",
                        engine=inst.engine,
                        sync_info=mybir.SyncInfo(on_wait=[w], on_update=[]),
                    )
                    for k, w in enumerate(waits[:-1])
                ]
                si.on_wait = [waits[-1]]
                for k, nop in enumerate(new_nops):
                    insts.insert(i + k, nop)
                i += len(new_nops) + 1
                n_split += 1
    return n_split


def _strip_barriers(nc):
    """Remove the Tile entry all-engine barrier (safe: no const-pool reads —
    all cross-engine deps are explicit semaphores) and the second exit
    barrier after the semaphore-reset ISA op (safe: engines halt after it and
    the runtime waits for all halts before any re-run)."""
    for fn in nc.m.functions:
        for b in fn.blocks:
            insts = b.instructions
            if b.name == "main":
                keep = [i for i in insts
                        if str(i.opcode) not in ("Drain", "EventSemaphore")]
                insts[:] = keep
            elif b.name.endswith("_end"):
                last_isa = max((k for k, i in enumerate(insts)
                                if str(i.opcode) == "ISA"), default=None)
                if last_isa is not None:
                    insts[:] = insts[:last_isa + 1]


def _offsets_and_weights():
    idx = np.stack(np.nonzero(np.ones(SELEM_SHAPE)), axis=-1)      # (243, 3)
    disp = idx - np.asarray(CENTRE)
    strides = np.array([H * W3, W3, 1])
    offsets = disp @ strides                                        # (243,)
    dist = np.linalg.norm(disp.astype(np.float64), axis=1)
    weights = (dist / dist.max() - 1.0).astype(np.float32)          # (243,)
    return offsets.astype(np.int64), weights


def _build_nc():
    nc = bass.Bass()
    f32 = mybir.dt.float32
    u8 = mybir.dt.uint8
    big = nc.dram_tensor("big", (P, WTOT), f32, kind="ExternalInput")
    out = nc.dram_tensor("out", (5, 1), f32, kind="ExternalOutput")
    Ln = mybir.ActivationFunctionType.Ln
    Ident = mybir.ActivationFunctionType.Identity
    Al = mybir.AluOpType

    with TileContext(nc) as tc:
        with tc.tile_pool(name="pool", bufs=1) as pool, \
             tc.tile_pool(name="psum", bufs=1, space="PSUM") as psum_pool:
            o = pool.tile([P, 5], f32)
            ones_col = pool.tile([P, 1], f32)
            nc.vector.memset(ones_col[:], 1.0)
            warm = pool.tile([P, 1], f32)
            nc.gpsimd.memset(warm[:], 0.5)
            big_t = pool.tile([P, WTOT], f32)
            # 4 partition-group DMAs on parallel rings, 2 per trigger engine
            nc.sync.dma_start(out=big_t[0:32, :], in_=big[0:32, :])
            nc.scalar.dma_start(out=big_t[32:64, :], in_=big[32:64, :])
            nc.sync.dma_start(out=big_t[64:96, :], in_=big[64:96, :])
            nc.scalar.dma_start(out=big_t[96:128, :], in_=big[96:128, :])
            # prewarm the Ln table while DMAs are in flight (bias = warm
            # itself: no const-pool read, no data dependency)
            nc.scalar.activation(warm[:], warm[:], Ln, bias=warm[:, 0:1])
            x_all = big_t[:, 0:XW]
            a_v = big_t[:, XW:XW + AW]
            zeros = big_t[:, XW + AW:XW + AW + 1]
            ones_b = big_t[:, XW + AW + 1:XW + AW + 2]
            t_u8 = big_t[:, XW + AW + BW:WTOT].bitcast(u8)   # (P, CH*F) u8
            junkv = pool.tile([P, 3 * F], f32)
            junks = pool.tile([P, F], f32)
            lnp_all = pool.tile([P, 3, F], f32)
            ln1p_all = pool.tile([P, 3, F], f32)
            for c in range(3):
                nc.scalar.activation(lnp_all[:, c, :],
                                     big_t[:, c * F:(c + 1) * F], Ln,
                                     bias=zeros)
            for c in range(3):
                nc.scalar.activation(ln1p_all[:, c, :],
                                     big_t[:, c * F:(c + 1) * F], Ln,
                                     bias=ones_b, scale=-1.0)
            # col2: dots over channels 0-2 in one pass (A broadcast along ch)
            a_b = a_v.rearrange("p (o f) -> p o f", o=1).broadcast_to((P, 3, F))
            x3v = x_all.rearrange("p (c f) -> p c f", c=CH)[:, 0:3, :]
            nc.vector.scalar_tensor_tensor(
                junkv[:].rearrange("p (c f) -> p c f", c=3), x3v, 0.0, a_b,
                Al.bypass, Al.mult, accum_out=o[:, 2:3])
            # col3: dot for channel 3
            nc.vector.scalar_tensor_tensor(
                junks[:], big_t[:, 3 * F:4 * F], 0.0, a_v,
                Al.bypass, Al.mult, accum_out=o[:, 3:4])
            # col0: sum(t012 * lnp012); col1: sum((t012-1) * ln1p012)
            t3v = t_u8[:, 0:3 * F]
            nc.vector.scalar_tensor_tensor(
                junkv[:], t3v, 0.0, lnp_all[:].rearrange("p c f -> p (c f)"),
                Al.bypass, Al.mult, accum_out=o[:, 0:1])
            nc.vector.scalar_tensor_tensor(
                junkv[:], t3v, 1.0, ln1p_all[:].rearrange("p c f -> p (c f)"),
                Al.subtract, Al.mult, accum_out=o[:, 1:2])
            # col4: n_cent partial = sum(t_3)
            nc.scalar.activation(junks[:], t_u8[:, 3 * F:4 * F], Ident,
                                 bias=zeros, accum_out=o[:, 4:5])
            # PE folds (128,5) -> (5,1) column sums; DVE copies PSUM->SBUF
            ps = psum_pool.tile([5, 1], f32)
            nc.tensor.matmul(ps[:], o[:], ones_col[:])
            o_small = pool.tile([5, 1], f32)
            nc.vector.tensor_copy(o_small[:], ps[:])
            nc.sync.dma_start(out=out[:, :], in_=o_small[:])
    _split_multi_waits(nc)
    _strip_barriers(nc)
    return nc


def _host_a_vector(cm):
    """Dense A with A[j] = sum_{centroid i, tap k: i+off_k=j} cm_i * w_k / cnt_i."""
    offsets, weights = _offsets_and_weights()
    A = np.zeros(N, dtype=np.float64)
    idx = np.nonzero(cm != 0.0)[0]
    for i in idx:
        ni = i + offsets
        valid = (ni >= 0) & (ni < N)
        cnt = float(valid.sum())
        A[ni[valid]] += (cm[i] / max(cnt, 1.0)) * weights[valid].astype(np.float64)
    return A.astype(np.float32)


def kernel(inputs: np.ndarray, targets: np.ndarray) -> np.ndarray:
    x_full = np.ascontiguousarray(np.asarray(inputs, dtype=np.float32).reshape(CH, N))
    t_full = np.ascontiguousarray(np.asarray(targets, dtype=np.float32).reshape(CH, N))

    A = _host_a_vector(t_full[3])

    in_maps = []
    for i in range(NCORES):
        sl = slice(i * CHUNK, (i + 1) * CHUNK)
        x_sh = x_full[:, sl].reshape(CH, P, F).transpose(1, 0, 2)   # (P,CH,F)
        t_sh = t_full[:, sl].reshape(CH, P, F).transpose(1, 0, 2)
        big = np.zeros((P, WTOT), dtype=np.float32)
        big[:, 0:XW] = x_sh.reshape(P, XW)
        big[:, XW:XW + AW] = A[sl].reshape(P, F)
        big[:, XW + AW + 1] = 1.0
        t_u8 = np.ascontiguousarray(t_sh.reshape(P, CH * F)).astype(np.uint8)
        big[:, XW + AW + BW:WTOT] = t_u8.view(np.float32)
        in_maps.append({"big": big})

    if "nc" not in _cache:
        _cache["nc"] = _build_nc()
    nc = _cache["nc"]

    trace = bool(int(os.environ.get("KERNEL_TRACE", "0")))
    res = bass_utils.run_bass_kernel_spmd(
        nc, in_maps, core_ids=list(range(NCORES)), trace=trace)
    kernel._last_results = res

    r = np.zeros(5, dtype=np.float64)
    for m in res.results:
        r += m["out"].astype(np.float64).ravel()

    # r: [sum(t*lnp), sum((t-1)*ln1p), dots012, dot3, ncent] over 3-ch groups
    loss = (r[1] - r[0]) / (3.0 * N)
    n_cent = max(r[4], 1.0)
    aff_pen = (r[2] / n_cent) / 3.0 * PHI
    cent_pen = (1.0 - r[3] / n_cent) * ETA
    return np.asarray(loss + aff_pen + cent_pen, dtype=np.float32)


# revision 16
# speedup vs baseline: 1.0604x; 1.0604x over previous
"""Trainium2 Bass kernel for nn_CentroidLoss (BCE + sparse-centroid selem similarity).

Full inputs in, full (scalar) output out. Sharding: flattened voxel axis N
split contiguously across 8 cores (one D-slice each).

Math: loss = mean_c BCE(x_c, t_c) + 0.5*mean(sims[:3]) + 0.5*(1-sims[3]) where
sims_c = (1/n_cent) * sum_i cm_i * (sum_k w_k valid x_c[i+off_k]) / cnt_i.
The centroid mask cm is ~0.01% dense, so the double sum is re-associated as
dot(x_c, A) with A[j] = sum_{i,k: i+off_k=j} cm_i * w_k / cnt_i, a sparse
scatter computed on host from the mask (~82*243 scalar ops).

Device (per core): ONE packed DRAM tensor (128 partitions x 4802 f32 =
x[4ch f32] | A[f32] | bias{0,1} | t[4ch u8]) loaded via 4 partition-group
DMAs (DMA cost here is per-descriptor, so fewest+fattest rows win, split
across rings for parallelism). ScalarE computes ln(x), ln(1-x) (table
prewarmed); VectorE does 4 fused multiply+row-sum reductions (channels
combined — only channel-sums are needed); PE folds the (128,5) partials to
(5,1) with a ones-column matmul; host combines 8x5 partials into the loss.
CHAN_WEIGHTS are all 1.0, so only channel-summed BCE/dot terms are needed.
"""

import os
import numpy as np

import concourse.bass as bass
import concourse.mybir as mybir
from concourse.tile import TileContext
from concourse import bass_utils

# ---- hardcoded problem geometry ----
D, H, W3 = 8, 320, 320
N = D * H * W3                     # 819200
NCORES = 8
CHUNK = N // NCORES                # 102400
P = 128
F = CHUNK // P                     # 800
CH = 4
EPS = 1e-7
ETA = 0.5
PHI = 0.5

SELEM_SHAPE = (3, 9, 9)
CENTRE = (1, 4, 4)

# packed-row layout (f32 columns)
XW = CH * F                        # 3200: x, channel-major
AW = F                             # 800: A
BW = 2                             # bias 0.0, 1.0
TW = CH * F // 4                   # 800: t as u8 bytes in f32 words
WTOT = XW + AW + BW + TW           # 4802

_cache = {}


def _split_multi_waits(nc):
    """This walrus build rejects >1 sync-wait per instruction ("Too many sync
    wait commands"). Tile coalesces waits; redistribute extras onto NoOps
    inserted immediately before, on the same engine (engine blocks on each
    wait in turn — semantics preserved)."""
    n_split = 0
    for fn in nc.m.functions:
        for b in fn.blocks:
            insts = b.instructions
            i = 0
            while i < len(insts):
                inst = insts[i]
                si = getattr(inst, 'sync_info', None)
                if si is None or not si.on_wait or len(si.on_wait) <= 1:
                    i += 1
                    continue
                waits = list(si.on_wait)
                new_nops = [
                    mybir.InstNoOp(
                        name=f"{inst.name}-waitsplit-{k}",
                        engine=inst.engine,
                        sync_info=mybir.SyncInfo(on_wait=[w], on_update=[]),
                    )
                    for k, w in enumerate(waits[:-1])
                ]
                si.on_wait = [waits[-1]]
                for k, nop in enumerate(new_nops):
                    insts.insert(i + k, nop)
                i += len(new_nops) + 1
                n_split += 1
    return n_split


def _strip_barriers(nc):
    """Remove the Tile entry all-engine barrier (safe: no const-pool reads —
    all cross-engine deps are explicit semaphores) and the second exit
    barrier after the semaphore-reset ISA op (safe: engines halt after it and
    the runtime waits for all halts before any re-run)."""
    for fn in nc.m.functions:
        for b in fn.blocks:
            insts = b.instructions
            if b.name == "main":
                keep = [i for i in insts
                        if str(i.opcode) not in ("Drain", "EventSemaphore")]
                insts[:] = keep
            elif b.name.endswith("_end"):
                last_isa = max((k for k, i in enumerate(insts)
                                if str(i.opcode) == "ISA"), default=None)
                if last_isa is not None:
                    insts[:] = insts[:last_isa + 1]


def _offsets_and_weights():
    idx = np.stack(np.nonzero(np.ones(SELEM_SHAPE)), axis=-1)      # (243, 3)
    disp = idx - np.asarray(CENTRE)
    strides = np.array([H * W3, W3, 1])
    offsets = disp @ strides                                        # (243,)
    dist = np.linalg.norm(disp.astype(np.float64), axis=1)
    weights = (dist / dist.max() - 1.0).astype(np.float32)          # (243,)
    return offsets.astype(np.int64), weights


def _build_nc():
    nc = bass.Bass()
    f32 = mybir.dt.float32
    u8 = mybir.dt.uint8
    x = nc.dram_tensor("x", (P, CH, F), f32, kind="ExternalInput")
    a = nc.dram_tensor("a", (P, F), f32, kind="ExternalInput")
    t = nc.dram_tensor("t", (P, CH * F), u8, kind="ExternalInput")
    out = nc.dram_tensor("out", (11, 1), f32, kind="ExternalOutput")
    Ln = mybir.ActivationFunctionType.Ln
    Ident = mybir.ActivationFunctionType.Identity
    Al = mybir.AluOpType

    with TileContext(nc) as tc:
        with tc.tile_pool(name="pool", bufs=1) as pool, \
             tc.tile_pool(name="psum", bufs=1, space="PSUM") as psum_pool:
            o = pool.tile([P, 11], f32)
            ones_col = pool.tile([P, 1], f32)
            nc.vector.memset(ones_col[:], 1.0)
            zero_b = pool.tile([P, 1], f32)
            nc.vector.memset(zero_b[:], 0.0)
            one_b = pool.tile([P, 1], f32)
            nc.vector.memset(one_b[:], 1.0)
            warm = pool.tile([P, 1], f32)
            nc.gpsimd.memset(warm[:], 0.5)
            # need-ordered DMAs; two HWDGE queue families share ~200GB/s
            a_t = pool.tile([P, F], f32)
            x_t = pool.tile([P, CH, F], f32)
            t_t = pool.tile([P, CH * F], u8)
            nc.sync.dma_start(out=a_t[:], in_=a[:, :])
            nc.scalar.dma_start(out=x_t[:, 2:3, :], in_=x[:, 2:3, :])
            nc.sync.dma_start(out=x_t[:, 0:1, :], in_=x[:, 0:1, :])
            nc.scalar.dma_start(out=t_t[:], in_=t[:, :])
            nc.sync.dma_start(out=x_t[:, 1:2, :], in_=x[:, 1:2, :])
            nc.scalar.dma_start(out=x_t[:, 3:4, :], in_=x[:, 3:4, :])
            # prewarm the Ln table while DMAs are in flight
            nc.scalar.activation(warm[:], warm[:], Ln, bias=warm[:, 0:1])
            junkv = pool.tile([P, F], f32)
            junks = pool.tile([P, F], f32)
            lnps, ln1ps = {}, {}
            for c in (2, 0):
                lnp_c = pool.tile([P, F], f32, name=f"lnp{c}")
                nc.scalar.activation(lnp_c[:], x_t[:, c, :], Ln,
                                     bias=zero_b[:])
                ln1p_c = pool.tile([P, F], f32, name=f"ln1p{c}")
                nc.scalar.activation(ln1p_c[:], x_t[:, c, :], Ln,
                                     bias=one_b[:], scale=-1.0)
                lnps[c], ln1ps[c] = lnp_c, ln1p_c
            # col10: n_cent partial = sum(t_3) — fills the ACT idle slot
            nc.scalar.activation(junks[:], t_t[:, 3 * F:4 * F], Ident,
                                 bias=zero_b[:], accum_out=o[:, 10:11])
            for c in (1,):
                lnp_c = pool.tile([P, F], f32, name=f"lnp{c}")
                nc.scalar.activation(lnp_c[:], x_t[:, c, :], Ln,
                                     bias=zero_b[:])
                ln1p_c = pool.tile([P, F], f32, name=f"ln1p{c}")
                nc.scalar.activation(ln1p_c[:], x_t[:, c, :], Ln,
                                     bias=one_b[:], scale=-1.0)
                lnps[c], ln1ps[c] = lnp_c, ln1p_c

            def dot(c):
                # col 6+c: sum(x_c * a)
                nc.vector.scalar_tensor_tensor(
                    junkv[:], x_t[:, c, :], 0.0, a_t[:],
                    Al.bypass, Al.mult, accum_out=o[:, 6 + c:7 + c])

            def bce(c):
                # col c: sum(t_c * lnp_c); col 3+c: sum((t_c-1) * ln1p_c)
                tc_v = t_t[:, c * F:(c + 1) * F]
                nc.vector.scalar_tensor_tensor(
                    junkv[:], tc_v, 0.0, lnps[c][:],
                    Al.bypass, Al.mult, accum_out=o[:, c:c + 1])
                nc.vector.scalar_tensor_tensor(
                    junkv[:], tc_v, 1.0, ln1ps[c][:],
                    Al.subtract, Al.mult, accum_out=o[:, 3 + c:4 + c])

            dot(2)
            dot(0)
            bce(2)
            bce(0)
            dot(1)
            dot(3)
            bce(1)
            # PE folds (128,11) -> (11,1) column sums; DVE copies PSUM->SBUF
            ps = psum_pool.tile([11, 1], f32)
            nc.tensor.matmul(ps[:], o[:], ones_col[:])
            o_small = pool.tile([11, 1], f32)
            nc.vector.tensor_copy(o_small[:], ps[:])
            nc.sync.dma_start(out=out[:, :], in_=o_small[:])
    _split_multi_waits(nc)
    _strip_barriers(nc)
    return nc


def _host_a_vector(cm):
    """Dense A with A[j] = sum_{centroid i, tap k: i+off_k=j} cm_i * w_k / cnt_i."""
    offsets, weights = _offsets_and_weights()
    A = np.zeros(N, dtype=np.float64)
    idx = np.nonzero(cm != 0.0)[0]
    for i in idx:
        ni = i + offsets
        valid = (ni >= 0) & (ni < N)
        cnt = float(valid.sum())
        A[ni[valid]] += (cm[i] / max(cnt, 1.0)) * weights[valid].astype(np.float64)
    return A.astype(np.float32)


def kernel(inputs: np.ndarray, targets: np.ndarray) -> np.ndarray:
    x_full = np.ascontiguousarray(np.asarray(inputs, dtype=np.float32).reshape(CH, N))
    t_full = np.ascontiguousarray(np.asarray(targets, dtype=np.float32).reshape(CH, N))

    A = _host_a_vector(t_full[3])

    in_maps = []
    for i in range(NCORES):
        sl = slice(i * CHUNK, (i + 1) * CHUNK)
        x_sh = x_full[:, sl].reshape(CH, P, F).transpose(1, 0, 2)   # (P,CH,F)
        t_sh = t_full[:, sl].reshape(CH, P, F).transpose(1, 0, 2)
        in_maps.append({
            "x": np.ascontiguousarray(x_sh),
            "a": np.ascontiguousarray(A[sl]).reshape(P, F),
            "t": np.ascontiguousarray(
                t_sh.reshape(P, CH * F)).astype(np.uint8),
        })

    if "nc" not in _cache:
        _cache["nc"] = _build_nc()
    nc = _cache["nc"]

    trace = bool(int(os.environ.get("KERNEL_TRACE", "0")))
    res = bass_utils.run_bass_kernel_spmd(
        nc, in_maps, core_ids=list(range(NCORES)), trace=trace)
    kernel._last_results = res

    r = np.zeros(11, dtype=np.float64)
    for m in res.results:
        r += m["out"].astype(np.float64).ravel()

    # cols: 0-2 sum(t_c*lnp_c), 3-5 sum((t_c-1)*ln1p_c), 6-9 dot_c, 10 ncent
    loss = (r[3:6].sum() - r[0:3].sum()) / (3.0 * N)
    n_cent = max(r[10], 1.0)
    aff_pen = (r[6:9].sum() / n_cent) / 3.0 * PHI
    cent_pen = (1.0 - r[9] / n_cent) * ETA
    return np.asarray(loss + aff_pen + cent_pen, dtype=np.float32)


# revision 17
# speedup vs baseline: 1.1524x; 1.0867x over previous
"""Trainium2 Bass kernel for nn_CentroidLoss (BCE + sparse-centroid selem similarity).

Full inputs in, full (scalar) output out. Sharding: flattened voxel axis N
split contiguously across 8 cores (one D-slice each).

Math: loss = mean_c BCE(x_c, t_c) + 0.5*mean(sims[:3]) + 0.5*(1-sims[3]) where
sims_c = (1/n_cent) * sum_i cm_i * (sum_k w_k valid x_c[i+off_k]) / cnt_i.
The centroid mask cm is ~0.01% dense, so the double sum is re-associated as
dot(x_c, A) with A[j] = sum_{i,k: i+off_k=j} cm_i * w_k / cnt_i, a sparse
scatter computed on host from the mask (~82*243 scalar ops). The device
streams x, t and A once (memory-roofline) and emits per-partition partial
sums; host combines the 8*128 partials into the scalar loss.
"""

import os
import numpy as np

import concourse.bass as bass
import concourse.mybir as mybir
from concourse.tile import TileContext
from concourse import bass_utils

# ---- hardcoded problem geometry ----
D, H, W = 8, 320, 320
N = D * H * W                      # 819200
NCORES = 8
CHUNK = N // NCORES                # 102400
P = 128
F = CHUNK // P                     # 800
CH = 4
EPS = 1e-7
ETA = 0.5
PHI = 0.5
CHAN_WEIGHTS = (1.0, 1.0, 1.0)

SELEM_SHAPE = (3, 9, 9)
CENTRE = (1, 4, 4)

_cache = {}


def _split_multi_waits(nc):
    """This walrus build rejects >1 sync-wait per instruction ("Too many sync
    wait commands"). Tile coalesces waits; redistribute extras onto NoOps
    inserted immediately before, on the same engine (engine blocks on each
    wait in turn — semantics preserved)."""
    n_split = 0
    for fn in nc.m.functions:
        for b in fn.blocks:
            insts = b.instructions
            i = 0
            while i < len(insts):
                inst = insts[i]
                si = getattr(inst, 'sync_info', None)
                if si is None or not si.on_wait or len(si.on_wait) <= 1:
                    i += 1
                    continue
                waits = list(si.on_wait)
                new_nops = [
                    mybir.InstNoOp(
                        name=f"{inst.name}-waitsplit-{k}",
                        engine=inst.engine,
                        sync_info=mybir.SyncInfo(on_wait=[w], on_update=[]),
                    )
                    for k, w in enumerate(waits[:-1])
                ]
                si.on_wait = [waits[-1]]
                for k, nop in enumerate(new_nops):
                    insts.insert(i + k, nop)
                i += len(new_nops) + 1
                n_split += 1
    return n_split


def _offsets_and_weights():
    idx = np.stack(np.nonzero(np.ones(SELEM_SHAPE)), axis=-1)      # (243, 3)
    disp = idx - np.asarray(CENTRE)
    strides = np.array([H * W, W, 1])
    offsets = disp @ strides                                        # (243,)
    dist = np.linalg.norm(disp.astype(np.float64), axis=1)
    weights = (dist / dist.max() - 1.0).astype(np.float32)          # (243,)
    return offsets.astype(np.int64), weights


def _strip_barriers(nc):
    """Remove the Tile entry all-engine barrier (safe: no const-pool reads —
    all cross-engine deps are explicit semaphores) and the second exit
    barrier after the semaphore-reset ISA op (safe: engines halt after it and
    the runtime waits for all halts before any re-run)."""
    for fn in nc.m.functions:
        for b in fn.blocks:
            insts = b.instructions
            if b.name == "main":
                keep = [i for i in insts
                        if str(i.opcode) not in ("Drain", "EventSemaphore")]
                insts[:] = keep
            elif b.name.endswith("_end"):
                last_isa = max((k for k, i in enumerate(insts)
                                if str(i.opcode) == "ISA"), default=None)
                if last_isa is not None:
                    insts[:] = insts[:last_isa + 1]


def _build_nc():
    nc = bass.Bass()
    f32 = mybir.dt.float32
    bf16 = mybir.dt.bfloat16
    # channel-interleaved per-core layout: partition-major, then channel.
    # `a` carries two extra columns (0.0, 1.0) used as activation bias APs so
    # the kernel never touches the const pool (required for barrier strip).
    x = nc.dram_tensor("x", (P, CH, F), f32, kind="ExternalInput")
    t = nc.dram_tensor("t", (P, CH, F), bf16, kind="ExternalInput")
    a = nc.dram_tensor("a", (P, F + 2), f32, kind="ExternalInput")
    out = nc.dram_tensor("out", (P, 12), f32, kind="ExternalOutput")
    Ln = mybir.ActivationFunctionType.Ln
    Ident = mybir.ActivationFunctionType.Identity
    Al = mybir.AluOpType

    with TileContext(nc) as tc:
        with tc.tile_pool(name="pool", bufs=1) as pool:
            o = pool.tile([P, 12], f32)
            warm = pool.tile([P, 1], f32)
            nc.gpsimd.memset(warm[:], 0.5)
            # Two HWDGE queues (one per trigger engine), each ~195GB/s and
            # FIFO — arrivals follow issue order. Balance bytes and order by
            # first need: qSP: a, x0, x1, t01; qAct: x2, x3, t23.
            a_t = pool.tile([P, F + 2], f32)
            x_t = pool.tile([P, CH, F], f32)
            t_t = pool.tile([P, CH, F], bf16)
            nc.sync.dma_start(out=a_t[:], in_=a[:, :])
            nc.scalar.dma_start(out=x_t[:, 2:3, :], in_=x[:, 2:3, :])
            nc.sync.dma_start(out=x_t[:, 0:1, :], in_=x[:, 0:1, :])
            nc.scalar.dma_start(out=x_t[:, 3:4, :], in_=x[:, 3:4, :])
            nc.sync.dma_start(out=x_t[:, 1:2, :], in_=x[:, 1:2, :])
            nc.scalar.dma_start(out=t_t[:, 2:4, :], in_=t[:, 2:4, :])
            nc.sync.dma_start(out=t_t[:, 0:2, :], in_=t[:, 0:2, :])
            # prewarm the Ln table while DMAs are in flight (bias = warm
            # itself: no const-pool read, no data dependency)
            nc.scalar.activation(warm[:], warm[:], Ln, bias=warm[:, 0:1])
            zeros = a_t[:, F:F + 1]
            ones = a_t[:, F + 1:F + 2]
            junkv = pool.tile([P, F], f32)
            junks = pool.tile([P, F], f32)
            lnps, ln1ps = {}, {}
            for c in (2, 0, 1):          # x2 arrives first (qAct head)
                lnp_c = pool.tile([P, F], f32, name=f"lnp{c}")
                nc.scalar.activation(lnp_c[:], x_t[:, c, :], Ln, bias=zeros)
                ln1p_c = pool.tile([P, F], f32, name=f"ln1p{c}")
                nc.scalar.activation(ln1p_c[:], x_t[:, c, :], Ln,
                                     bias=ones, scale=-1.0)
                lnps[c], ln1ps[c] = lnp_c, ln1p_c
            # vector, in expected data-readiness order
            def dot(c):
                nc.vector.scalar_tensor_tensor(
                    junkv[:], x_t[:, c, :], 0.0, a_t[:, 0:F],
                    Al.bypass, Al.mult, accum_out=o[:, 6 + c:7 + c])

            def bce(c):
                # col c: sum(t_c * ln p)
                nc.vector.scalar_tensor_tensor(
                    junkv[:], t_t[:, c, :], 0.0, lnps[c][:],
                    Al.bypass, Al.mult, accum_out=o[:, c:c + 1])
                # col 3+c: sum((t_c - 1) * ln(1-p))
                nc.vector.scalar_tensor_tensor(
                    junkv[:], t_t[:, c, :], 1.0, ln1ps[c][:],
                    Al.subtract, Al.mult, accum_out=o[:, 3 + c:4 + c])

            dot(2)
            dot(3)
            dot(0)
            dot(1)
            bce(2)
            bce(0)
            bce(1)
            # scalar: col 10 = n_cent partial = sum(t_3)
            nc.scalar.activation(junks[:], t_t[:, 3, :], Ident, bias=zeros,
                                 accum_out=o[:, 10:11])
            nc.sync.dma_start(out=out[:, :], in_=o[:, :])
    _split_multi_waits(nc)
    _strip_barriers(nc)
    return nc


def _host_a_vector(cm):
    """Dense A with A[j] = sum_{centroid i, tap k: i+off_k=j} cm_i * w_k / cnt_i."""
    offsets, weights = _offsets_and_weights()
    A = np.zeros(N, dtype=np.float64)
    idx = np.nonzero(cm != 0.0)[0]
    for i in idx:
        ni = i + offsets
        valid = (ni >= 0) & (ni < N)
        cnt = float(valid.sum())
        A[ni[valid]] += (cm[i] / max(cnt, 1.0)) * weights[valid].astype(np.float64)
    return A.astype(np.float32)


def kernel(inputs: np.ndarray, targets: np.ndarray) -> np.ndarray:
    x_full = np.ascontiguousarray(np.asarray(inputs, dtype=np.float32).reshape(CH, N))
    t_full = np.ascontiguousarray(np.asarray(targets, dtype=np.float32).reshape(CH, N))

    A = _host_a_vector(t_full[3])

    import ml_dtypes
    in_maps = []
    for i in range(NCORES):
        sl = slice(i * CHUNK, (i + 1) * CHUNK)
        x_sh = x_full[:, sl].reshape(CH, P, F).transpose(1, 0, 2)
        t_sh = t_full[:, sl].reshape(CH, P, F).transpose(1, 0, 2)
        a_sh = np.zeros((P, F + 2), dtype=np.float32)
        a_sh[:, :F] = A[sl].reshape(P, F)
        a_sh[:, F + 1] = 1.0
        in_maps.append({
            "x": np.ascontiguousarray(x_sh),
            "t": np.ascontiguousarray(t_sh).astype(ml_dtypes.bfloat16),
            "a": a_sh,
        })

    if "nc" not in _cache:
        _cache["nc"] = _build_nc()
    nc = _cache["nc"]

    trace = bool(int(os.environ.get("KERNEL_TRACE", "0")))
    res = bass_utils.run_bass_kernel_spmd(
        nc, in_maps, core_ids=list(range(NCORES)), trace=trace)
    kernel._last_results = res

    r = np.zeros(12, dtype=np.float64)
    for m in res.results:
        r += m["out"].astype(np.float64).sum(axis=0)

    sum_bce = r[3:6] - r[0:3]                 # sum of -(t lnp + (1-t) ln1p)
    chan_losses = sum_bce / N * np.asarray(CHAN_WEIGHTS, dtype=np.float64)
    loss = chan_losses.mean()
    n_cent = max(r[10], 1.0)
    sims = r[6:10] / n_cent
    result = loss + sims[:3].mean() * PHI + (1.0 - sims[3]) * ETA
    return np.asarray(result, dtype=np.float32)
